# revision 4
# baseline (speedup 1.0000x reference)
"""Trainium2 Bass kernel for nn_CostVolume (SpatialCorrelationSampler-style).

out[b, dy*9+dx, y, x] = sum_c feat1[b,c,y,x] * feat2_pad[b,c,y+dy,x+dx]
with feat2 zero-padded by 4 on H/W, dy/dx in [0,9), B=4, C=256, H=W=96.

Sharding (8 cores): core = (b, half) -- batch x H-half (48 rows each).

Device algorithm: 2D-tiled gram blocks. Per (y-block of 16, x-tile of 8):
  stationary = f1 tile [128c, 128=(16y x 8x)]  (128-col => FWL enabled)
  moving     = f2 window [128c, 384=(24r x 16x')], 1 MM per C-half
  psum[m=(yi,xx), n=r_rel*16+x_rel] accumulated over 2 C-halves.
Two x-tiles share a 2-bank PSUM tile. Staging copies crop each
32-partition group g to its union window [64g, 64g+192) and cast to a
dense bf16 stage [128, 12*192] per blk -- so output DMAs are 3 full-speed
contiguous slabs ([128, 1152] x 2 per blk) totalling 1.77MB/core.
The host strips the 81 displacements per pixel via a gather.

Total HBM bytes/core ~7.1MB (f1 2.36 + f2 2.98 + out 1.77); the kernel
is global-DMA-bandwidth-bound, so everything else hides under the load.
Warmup matmuls bridge the PE p-state ramp until real data lands.
"""

import numpy as np
import ml_dtypes

import concourse.bacc as bacc
import concourse.mybir as mybir
from concourse.ap import AP
from concourse.tile import TileContext
from concourse.bass_utils import run_bass_kernel_spmd

B, C, H, W = 4, 256, 96, 96
D = 4
P = 2 * D + 1     # 9 displacements per axis
HH = H // 2       # 48 rows per core
NB = 3            # y-blocks of TY
TY, TX = 16, 8
NXT = W // TX     # 12 x-tiles
XW = TX + 8       # 16-col f2 window per tile
F2R = HH + 8      # 56 padded f2 rows per core
F2W = W + 8       # 104 padded f2 cols
F2HP = F2R * F2W  # 5824 f2 elems per (partition, ch)
F1HP = NB * NXT * 128  # 4608 f1 elems per (partition, ch)
RL = 192          # per-32-partition-group union window
STW = NXT * RL    # staged cols per block: 2304

F32 = mybir.dt.float32
BF16 = mybir.dt.bfloat16

_CACHED = {}


def _build_nc():
    nc = bacc.Bacc()
    f1 = nc.declare_dram_parameter("f1", [128, 2, NB, 1536], BF16, isOutput=False)
    f2 = nc.declare_dram_parameter("f2", [128, 2, F2HP], BF16, isOutput=False)
    out = nc.declare_dram_parameter("o", [NB, 2, 128, STW // 2], BF16, isOutput=True)

    with TileContext(nc) as tc:
        with (
            tc.tile_pool(name="w", bufs=1) as wp,
            tc.tile_pool(name="m", bufs=1) as mp,
            tc.tile_pool(name="st", bufs=3) as stp,
            tc.tile_pool(name="ps", bufs=3, space="PSUM") as psp,
            tc.tile_pool(name="wu", bufs=1, space="PSUM") as wup,
        ):
            # PE warmup: dummy matmuls on a memset tile while inputs load.
            cst = wp.tile([128, 512], BF16, tag="cst", name="cst")
            nc.vector.memset(cst[:, :], 0)
            wups = wup.tile([128, 512], F32, tag="wups", name="wups")
            for _ in range(16):
                nc.tensor.matmul(
                    wups[:, 0:256], lhsT=cst[:, 0:128], rhs=cst[:, 0:256],
                    start=True, stop=True,
                )

            f1t = wp.tile([128, 2, F1HP], BF16, tag="f1t", name="f1t")
            f2t = mp.tile([128, 2, F2HP], BF16, tag="f2t", name="f2t")

            # Input DMAs: blk0-critical chunks first on each queue.
            A = 24 * F2W
            E = F2HP
            nc.scalar.dma_start(out=f1t[:, 0, 0:1536], in_=f1[:, 0, 0, :])
            nc.scalar.dma_start(out=f1t[:, 1, 0:1536], in_=f1[:, 1, 0, :])
            nc.scalar.dma_start(out=f1t[:, :, 1536:3072], in_=f1[:, :, 1, :])
            nc.scalar.dma_start(out=f1t[:, :, 3072:4608], in_=f1[:, :, 2, :])
            nc.sync.dma_start(out=f2t[:, 0, 0:A], in_=f2[:, 0, 0:A])
            nc.sync.dma_start(out=f2t[:, 1, 0:A], in_=f2[:, 1, 0:A])
            nc.sync.dma_start(out=f2t[:, 0, A:E], in_=f2[:, 0, A:E])
            nc.gpsimd.dma_start(out=f2t[:, 1, A:E], in_=f2[:, 1, A:E])

            ps_row = None
            st_row = None
            out_engines = [nc.gpsimd, nc.sync]
            oi = 0
            for blk in range(NB):
                st = stp.tile([128, STW], BF16, tag="st", name="st")
                if st_row is None:
                    st_row = st[:, 0:1].ap[0][0]
                for pair in range(NXT // 2):
                    ps = psp.tile([128, 1024], F32, tag="ps", name="ps")
                    if ps_row is None:
                        ps_row = ps[:, 0:1].ap[0][0]
                    for sub in range(2):
                        xt = 2 * pair + sub
                        for ch in range(2):
                            lhsT = AP(
                                tensor=f1t.tensor,
                                offset=f1t.offset + ch * F1HP + blk * 1536
                                + xt * 128,
                                ap=[[2 * F1HP, 128], [1, 128]],
                            )
                            rhs = AP(
                                tensor=f2t.tensor,
                                offset=f2t.offset + ch * F2HP
                                + 16 * blk * F2W + 8 * xt,
                                ap=[[2 * F2HP, 128], [F2W, 24], [1, XW]],
                            )
                            nc.tensor.matmul(
                                ps[:, 512 * sub : 512 * sub + 384],
                                lhsT=lhsT,
                                rhs=rhs,
                                start=(ch == 0),
                                stop=(ch == 1),
                            )
                    # crop-copy: group g of 32 partitions keeps its union
                    # window [64g, 64g+192) of both x-tiles in the pair,
                    # cast f32->bf16 into the dense stage.
                    for g, eng in ((0, nc.vector), (1, nc.scalar),
                                   (2, nc.vector), (3, nc.scalar)):
                        src = AP(
                            tensor=ps.tensor,
                            offset=ps.offset + 32 * g * ps_row + 64 * g,
                            ap=[[ps_row, 32], [512, 2], [1, RL]],
                        )
                        dst = AP(
                            tensor=st.tensor,
                            offset=st.offset + 32 * g * st_row
                            + 2 * pair * RL,
                            ap=[[st_row, 32], [RL, 2], [1, RL]],
                        )
                        if eng is nc.scalar:
                            eng.copy(out=dst, in_=src)
                        else:
                            eng.tensor_copy(dst, src)
                    if pair == 2 or pair == 5:
                        h = 0 if pair == 2 else 1
                        src = st[:, h * (STW // 2) : (h + 1) * (STW // 2)]
                        out_engines[oi % 2].dma_start(out=out[blk, h], in_=src)
                        oi += 1
    nc.finalize()
    return nc


def kernel(feat1: np.ndarray, feat2: np.ndarray) -> np.ndarray:
    feat1 = np.ascontiguousarray(np.asarray(feat1, dtype=np.float32))
    feat2 = np.ascontiguousarray(np.asarray(feat2, dtype=np.float32))

    if "nc" not in _CACHED:
        _CACHED["nc"] = _build_nc()
    nc = _CACHED["nc"]

    core_ids = list(range(8))
    in_maps = []
    for core in core_ids:
        b, half = divmod(core, 2)
        f1h = feat1[b][:, half * HH : half * HH + HH, :]  # [256, 48, 96]
        # [c, blk, yi, xt, xx] -> [cl, ch, blk, (xt yi xx)]
        f1td = (
            f1h.reshape(256, NB, TY, NXT, TX)
            .transpose(0, 1, 3, 2, 4)
            .reshape(2, 128, NB, NXT * 128)
            .transpose(1, 0, 2, 3)
        )
        f2p = np.pad(feat2[b], ((0, 0), (D, D), (D, D)))[
            :, half * HH : half * HH + F2R, :
        ]  # [256, 56, 104]
        f2td = f2p.reshape(2, 128, F2HP).transpose(1, 0, 2)
        in_maps.append(
            {
                "f1": np.ascontiguousarray(f1td.astype(ml_dtypes.bfloat16)),
                "f2": np.ascontiguousarray(f2td.astype(ml_dtypes.bfloat16)),
            }
        )

    res = run_bass_kernel_spmd(nc, in_maps, core_ids)

    # gather index: stage col for pixel (yi, xx), disp (dy,dx) is
    # 16*(yi%4) + xx + 16*dy + dx within that pixel's x-tile 192-block.
    kidx = (
        16 * (np.arange(16) % 4)[:, None, None, None]
        + np.arange(8)[None, :, None, None]
        + 16 * np.arange(9)[None, None, :, None]
        + np.arange(9)[None, None, None, :]
    ).reshape(16, 8, 81)  # [yi, xx, k]

    out = np.empty((B, P * P, H, W), np.float32)
    for core in core_ids:
        b, half = divmod(core, 2)
        o = res.results[core]["o"]  # [NB, 2, 128, 1152] bf16
        v = (
            np.ascontiguousarray(o)
            .astype(np.float32)
            .reshape(NB, 2, 128, 6, RL)
            .transpose(0, 2, 1, 3, 4)
            .reshape(NB, 16, 8, NXT, RL)
        )  # [blk, yi, xx, xt, w]
        g = np.take_along_axis(
            v, kidx[None, :, :, None, :], axis=-1
        )  # [blk, yi, xx, xt, 81]
        core_out = g.transpose(4, 0, 1, 3, 2).reshape(P * P, HH, W)
        out[b, :, half * HH : half * HH + HH, :] = core_out
    return out


# revision 5
# speedup vs baseline: 1.2703x; 1.2703x over previous
"""Trainium2 Bass kernel for nn_CostVolume (SpatialCorrelationSampler-style).

out[b, dy*9+dx, y, x] = sum_c feat1[b,c,y,x] * feat2_pad[b,c,y+dy,x+dx]
with feat2 zero-padded by 4 on H/W, dy/dx in [0,9), B=4, C=256, H=W=96.

Sharding (8 cores): core = (b, half) -- batch x H-half (48 rows each).

Device algorithm: 2D-tiled gram blocks. Per (y-block of 16, x-tile of 8):
  stationary = f1 tile [128c, 128=(16y x 8x)]  (128-col => FWL enabled)
  moving     = f2 window [128c, 384=(24r x 16x')], 1 MM per C-half
  psum[m=(yi,xx), n=r_rel*16+x_rel] accumulated over 2 C-halves.
Two x-tiles share a 2-bank PSUM tile. Staging copies crop each
64-partition group g to its union window [128g, 128g+264) and cast to a
dense bf16 stage [128, 12*264] per blk (one copy per engine per pair) --
output DMAs are full-speed contiguous slabs totalling 2.43MB/core.
The host strips the 81 displacements per pixel via a gather.

Total HBM bytes/core ~7.1MB (f1 2.36 + f2 2.98 + out 1.77); the kernel
is global-DMA-bandwidth-bound, so everything else hides under the load.
Warmup matmuls bridge the PE p-state ramp until real data lands.
"""

import numpy as np
import ml_dtypes

import concourse.bacc as bacc
import concourse.mybir as mybir
from concourse.ap import AP
from concourse.tile import TileContext
from concourse.bass_utils import run_bass_kernel_spmd

B, C, H, W = 4, 256, 96, 96
D = 4
P = 2 * D + 1     # 9 displacements per axis
HH = H // 2       # 48 rows per core
NB = 3            # y-blocks of TY
TY, TX = 16, 8
NXT = W // TX     # 12 x-tiles
XW = TX + 8       # 16-col f2 window per tile
F2R = HH + 8      # 56 padded f2 rows per core
F2W = W + 8       # 104 padded f2 cols
F2HP = F2R * F2W  # 5824 f2 elems per (partition, ch)
F1HP = NB * NXT * 128  # 4608 f1 elems per (partition, ch)
RL = 264          # per-64-partition-group union window
STW = NXT * RL    # staged cols per block: 2304

F32 = mybir.dt.float32
BF16 = mybir.dt.bfloat16

_CACHED = {}


def _build_nc():
    nc = bacc.Bacc()
    f1 = nc.declare_dram_parameter("f1", [128, 2, NB, 1536], BF16, isOutput=False)
    f2 = nc.declare_dram_parameter("f2", [128, 2, F2HP], BF16, isOutput=False)
    out = nc.declare_dram_parameter("o", [NB, 2, 128, STW // 2], BF16, isOutput=True)

    with TileContext(nc) as tc:
        with (
            tc.tile_pool(name="w", bufs=1) as wp,
            tc.tile_pool(name="m", bufs=1) as mp,
            tc.tile_pool(name="st", bufs=3) as stp,
            tc.tile_pool(name="ps", bufs=3, space="PSUM") as psp,
            tc.tile_pool(name="wu", bufs=1, space="PSUM") as wup,
        ):
            # PE warmup: dummy matmuls on a memset tile while inputs load.
            cst = wp.tile([128, 512], BF16, tag="cst", name="cst")
            nc.vector.memset(cst[:, :], 0)
            wups = wup.tile([128, 512], F32, tag="wups", name="wups")
            for _ in range(16):
                nc.tensor.matmul(
                    wups[:, 0:256], lhsT=cst[:, 0:128], rhs=cst[:, 0:256],
                    start=True, stop=True,
                )

            f1t = wp.tile([128, 2, F1HP], BF16, tag="f1t", name="f1t")
            f2t = mp.tile([128, 2, F2HP], BF16, tag="f2t", name="f2t")

            # Input DMAs: blk0-critical chunks first on each queue.
            A = 24 * F2W
            E = F2HP
            nc.scalar.dma_start(out=f1t[:, 0, 0:1536], in_=f1[:, 0, 0, :])
            nc.scalar.dma_start(out=f1t[:, 1, 0:1536], in_=f1[:, 1, 0, :])
            nc.scalar.dma_start(out=f1t[:, :, 1536:3072], in_=f1[:, :, 1, :])
            nc.scalar.dma_start(out=f1t[:, :, 3072:4608], in_=f1[:, :, 2, :])
            nc.sync.dma_start(out=f2t[:, 0, 0:A], in_=f2[:, 0, 0:A])
            nc.sync.dma_start(out=f2t[:, 1, 0:A], in_=f2[:, 1, 0:A])
            nc.sync.dma_start(out=f2t[:, 0, A:E], in_=f2[:, 0, A:E])
            nc.gpsimd.dma_start(out=f2t[:, 1, A:E], in_=f2[:, 1, A:E])

            ps_row = None
            st_row = None
            out_engines = [nc.gpsimd, nc.sync]
            oi = 0
            for blk in range(NB):
                st = stp.tile([128, STW], BF16, tag="st", name="st")
                if st_row is None:
                    st_row = st[:, 0:1].ap[0][0]
                for pair in range(NXT // 2):
                    ps = psp.tile([128, 1024], F32, tag="ps", name="ps")
                    if ps_row is None:
                        ps_row = ps[:, 0:1].ap[0][0]
                    for sub in range(2):
                        xt = 2 * pair + sub
                        for ch in range(2):
                            lhsT = AP(
                                tensor=f1t.tensor,
                                offset=f1t.offset + ch * F1HP + blk * 1536
                                + xt * 128,
                                ap=[[2 * F1HP, 128], [1, 128]],
                            )
                            rhs = AP(
                                tensor=f2t.tensor,
                                offset=f2t.offset + ch * F2HP
                                + 16 * blk * F2W + 8 * xt,
                                ap=[[2 * F2HP, 128], [F2W, 24], [1, XW]],
                            )
                            nc.tensor.matmul(
                                ps[:, 512 * sub : 512 * sub + 384],
                                lhsT=lhsT,
                                rhs=rhs,
                                start=(ch == 0),
                                stop=(ch == 1),
                            )
                    # crop-copy: group g of 32 partitions keeps its union
                    # window [64g, 64g+192) of both x-tiles in the pair,
                    # cast f32->bf16 into the dense stage.
                    for g, eng in ((0, nc.vector), (1, nc.scalar)):
                        src = AP(
                            tensor=ps.tensor,
                            offset=ps.offset + 64 * g * ps_row + 128 * g,
                            ap=[[ps_row, 64], [512, 2], [1, RL]],
                        )
                        dst = AP(
                            tensor=st.tensor,
                            offset=st.offset + 64 * g * st_row
                            + 2 * pair * RL,
                            ap=[[st_row, 64], [RL, 2], [1, RL]],
                        )
                        if eng is nc.scalar:
                            eng.copy(out=dst, in_=src)
                        else:
                            eng.tensor_copy(dst, src)
                    if pair == 2 or pair == 5:
                        h = 0 if pair == 2 else 1
                        src = st[:, h * (STW // 2) : (h + 1) * (STW // 2)]
                        out_engines[oi % 2].dma_start(out=out[blk, h], in_=src)
                        oi += 1
    nc.finalize()
    return nc


def kernel(feat1: np.ndarray, feat2: np.ndarray) -> np.ndarray:
    feat1 = np.ascontiguousarray(np.asarray(feat1, dtype=np.float32))
    feat2 = np.ascontiguousarray(np.asarray(feat2, dtype=np.float32))

    if "nc" not in _CACHED:
        _CACHED["nc"] = _build_nc()
    nc = _CACHED["nc"]

    core_ids = list(range(8))
    in_maps = []
    for core in core_ids:
        b, half = divmod(core, 2)
        f1h = feat1[b][:, half * HH : half * HH + HH, :]  # [256, 48, 96]
        # [c, blk, yi, xt, xx] -> [cl, ch, blk, (xt yi xx)]
        f1td = (
            f1h.reshape(256, NB, TY, NXT, TX)
            .transpose(0, 1, 3, 2, 4)
            .reshape(2, 128, NB, NXT * 128)
            .transpose(1, 0, 2, 3)
        )
        f2p = np.pad(feat2[b], ((0, 0), (D, D), (D, D)))[
            :, half * HH : half * HH + F2R, :
        ]  # [256, 56, 104]
        f2td = f2p.reshape(2, 128, F2HP).transpose(1, 0, 2)
        in_maps.append(
            {
                "f1": np.ascontiguousarray(f1td.astype(ml_dtypes.bfloat16)),
                "f2": np.ascontiguousarray(f2td.astype(ml_dtypes.bfloat16)),
            }
        )

    res = run_bass_kernel_spmd(nc, in_maps, core_ids)

    # gather index: stage col for pixel (yi, xx), disp (dy,dx) is
    # 16*(yi%8) + xx + 16*dy + dx within that pixel's x-tile 264-block.
    kidx = (
        16 * (np.arange(16) % 8)[:, None, None, None]
        + np.arange(8)[None, :, None, None]
        + 16 * np.arange(9)[None, None, :, None]
        + np.arange(9)[None, None, None, :]
    ).reshape(16, 8, 81)  # [yi, xx, k]

    out = np.empty((B, P * P, H, W), np.float32)
    for core in core_ids:
        b, half = divmod(core, 2)
        o = res.results[core]["o"]  # [NB, 2, 128, 1584] bf16
        v = (
            np.ascontiguousarray(o)
            .astype(np.float32)
            .reshape(NB, 2, 128, 6, RL)
            .transpose(0, 2, 1, 3, 4)
            .reshape(NB, 16, 8, NXT, RL)
        )  # [blk, yi, xx, xt, w]
        g = np.take_along_axis(
            v, kidx[None, :, :, None, :], axis=-1
        )  # [blk, yi, xx, xt, 81]
        core_out = g.transpose(4, 0, 1, 3, 2).reshape(P * P, HH, W)
        out[b, :, half * HH : half * HH + HH, :] = core_out
    return out
